# revision 1
# baseline (speedup 1.0000x reference)
"""Trainium2 Bass kernel for the DGCL GNN (3 GIN conv layers + 8-factor
disentangled head + global add pool).

Self-contained: host-side numpy preprocessing (graph partitioning /
weight packing), an SPMD Bass/Tile device program for 8 NeuronCores, and
the gather/unshard glue.

Structure of the computation (mathematically identical to the reference):
  - The K=8 disentangled head factors share the same edge aggregation, and
    their per-factor MLPs concatenate into [128,128] dense / block-diagonal
    matmuls.  So the network is 5 uniform layers:
        z = h + scatter_add(gather(h, src), dst)
        v = relu(z @ W1 + b1) @ W2 + b2
        h' = BN(v) (+ relu for layers 0,1,3)
    followed by a per-graph add-pool.
  - Nodes (and their incoming edges) are sharded contiguously across the 8
    cores.  Edge gathers read a bf16 replica of h from local DRAM via
    dma_gather (int16 indices -> lo/hi half split); aggregation happens as
    bf16 one-hot matmuls accumulating in fp32 PSUM.  The self term is added
    in fp32 from an SBUF-resident transposed copy of the core's own shard.
    The h replica is refreshed each layer with an AllGather; BN statistics
    and the pooled output use AllReduce.
"""

import math
from contextlib import ExitStack

import numpy as np

import concourse.bacc as bacc
import concourse.bass as bass
import concourse.mybir as mybir
import concourse.tile as tile
from concourse.bass_utils import run_bass_kernel_spmd
from concourse.masks import make_identity

P = 128
F = 128
GOUT = 512          # output graph rows (harness G = 512)
BN_EPS = 1e-5
RELU_AFTER = [True, True, False, True, False]
f32 = mybir.dt.float32
bf16 = mybir.dt.bfloat16
i16 = mybir.dt.int16


class Cfg:
    def __init__(self, N, C, Th, group_blocks=5):
        self.N = N                      # real node count
        self.C = C                      # cores
        self.Nshard = -(-N // (C * P)) * P
        self.Np = self.Nshard * C
        self.B = self.Nshard // P       # dst blocks per core
        self.Th = Th                    # 128-edge tiles per block-half
        self.Gb = group_blocks          # blocks per gather-call group
        self.n_groups = -(-self.B // group_blocks)
        self.H = self.Np // 2
        self.total_tiles = self.B * 2 * Th
        self.tile_base = None           # filled by prep_host / fill_groups
        self.nb_of_group = None
        self.no_cc = False
        self.layers = (0, 1, 2, 3, 4)   # which layer bodies to emit
        self.skip_agg = 0               # 1: no onehot/agg-mm; 2: no gathers too

    def fill_groups(self):
        tb, bases, nbs = 0, [], []
        for g in range(self.n_groups):
            nb = min((g + 1) * self.Gb, self.B) - g * self.Gb
            bases.append(tb)
            nbs.append(nb)
            tb += 2 * nb * self.Th
        self.tile_base, self.nb_of_group = bases, nbs
        return self


# ----------------------------------------------------------------------------
# Host-side preprocessing
# ----------------------------------------------------------------------------

def _blockdiag(W):
    K_, d_, _ = W.shape
    out = np.zeros((K_ * d_, K_ * d_), np.float32)
    for k in range(K_):
        out[k * d_:(k + 1) * d_, k * d_:(k + 1) * d_] = W[k]
    return out


def prep_host(x, edge_index, batch, params, C=8, group_blocks=5):
    """Returns (cfg, in_maps) ready for the device program."""
    x = np.asarray(x, np.float32)
    N = x.shape[0]
    src = np.asarray(edge_index[0], np.int64)
    dst = np.asarray(edge_index[1], np.int64)
    batch = np.asarray(batch, np.int64)

    order = np.argsort(dst, kind="stable")
    s_sorted = src[order]
    d_sorted = dst[order]

    gblock = d_sorted // P                      # global dst block id
    slot = (d_sorted % P).astype(np.float32)

    NshardP = -(-N // (C * P)) * P
    B = NshardP // P
    Np = NshardP * C
    H = Np // 2
    assert H <= 32767 and Np - H <= 32767, "half-table exceeds int16 range"

    # split each block's edge list by source half (int16 index limit)
    is_hi = (s_sorted >= H).astype(np.int64)
    halfkey = gblock * 2 + is_hi
    counts2 = np.bincount(halfkey, minlength=C * B * 2)
    Th = max(1, int(math.ceil(counts2.max() / P)))
    cfg = Cfg(N, C, Th, group_blocks).fill_groups()
    Gb = group_blocks

    total_tiles = cfg.total_tiles
    idx16 = np.zeros((C, 16, total_tiles * 8), np.int16)
    slot_arr = np.full((C, 128, total_tiles), 999.0, np.float32)

    order2 = np.argsort(halfkey, kind="stable")
    s2 = s_sorted[order2]
    slot2 = slot[order2]
    startpos = np.zeros(C * B * 2 + 1, np.int64)
    startpos[1:] = np.cumsum(counts2)

    for c in range(C):
        for b in range(B):
            g = b // Gb
            bi = b - g * Gb
            nb = cfg.nb_of_group[g]
            for hf in range(2):
                k = (c * B + b) * 2 + hf
                cnt = counts2[k]
                if cnt == 0:
                    continue
                sl = slice(startpos[k], startpos[k + 1])
                j = np.arange(cnt)
                call_tile0 = cfg.tile_base[g] + (nb * Th if hf else 0)
                gt = call_tile0 + bi * Th + j // P
                slot_arr[c, j % P, gt] = slot2[sl]
                vals = s2[sl] - (H if hf else 0)
                gcol = (call_tile0 + bi * Th) * 8 + j // 16
                idx16[c, j % 16, gcol] = vals.astype(np.int16)
    idx16 = np.tile(idx16, (1, 8, 1))           # replicate 16-row wrap x8

    # per-node batch selector (999 sentinel for padding nodes)
    ids = np.arange(Np)
    bsel = np.where(ids < N, batch[np.minimum(ids, N - 1)], 999).astype(np.float32)
    batchsel = bsel.reshape(C, B, P).transpose(0, 2, 1).copy()   # [C, P, B]

    npad = np.array(
        [max(0, (c + 1) * NshardP - max(N, c * NshardP)) for c in range(C)],
        np.float32,
    )

    # padded node features (zeros for pad rows) + bf16 gather replica
    x_pad = np.zeros((Np, F), np.float32)
    x_pad[:N] = x
    xb = x_pad.astype(mybir.dt.np(bf16))

    # --- weights ----------------------------------------------------------
    gc_W1 = np.asarray(params["gc_W1"], np.float32)
    gc_W2 = np.asarray(params["gc_W2"], np.float32)
    gc_b1 = np.asarray(params["gc_b1"], np.float32)
    gc_b2 = np.asarray(params["gc_b2"], np.float32)
    gc_g = np.asarray(params["gc_g"], np.float32)
    gc_be = np.asarray(params["gc_be"], np.float32)
    h0_W1 = np.asarray(params["h0_W1"], np.float32)
    h0_W2 = np.asarray(params["h0_W2"], np.float32)
    h1_W1 = np.asarray(params["h1_W1"], np.float32)
    h1_W2 = np.asarray(params["h1_W2"], np.float32)

    W1s = [gc_W1[0], gc_W1[1], gc_W1[2],
           h0_W1.transpose(1, 0, 2).reshape(F, F), _blockdiag(h1_W1)]
    W2s = [gc_W2[0], gc_W2[1], gc_W2[2], _blockdiag(h0_W2), _blockdiag(h1_W2)]
    b1s = [gc_b1[0], gc_b1[1], gc_b1[2],
           np.asarray(params["h0_b1"], np.float32).reshape(-1),
           np.asarray(params["h1_b1"], np.float32).reshape(-1)]
    b2s = [gc_b2[0], gc_b2[1], gc_b2[2],
           np.asarray(params["h0_b2"], np.float32).reshape(-1),
           np.asarray(params["h1_b2"], np.float32).reshape(-1)]
    gs = [gc_g[0], gc_g[1], gc_g[2],
          np.asarray(params["h0_g"], np.float32).reshape(-1),
          np.asarray(params["h1_g"], np.float32).reshape(-1)]
    bes = [gc_be[0], gc_be[1], gc_be[2],
           np.asarray(params["h0_be"], np.float32).reshape(-1),
           np.asarray(params["h1_be"], np.float32).reshape(-1)]

    wpack = np.concatenate(
        sum(([W1s[l], W2s[l]] for l in range(5)), []), axis=1
    ).astype(np.float32)                                        # [F, 1280]
    bpack = np.stack(
        sum(([b1s[l], b2s[l], gs[l], bes[l]] for l in range(5)), []), axis=1
    ).astype(np.float32)                                        # [F, 20]

    iota128 = np.tile(np.arange(P, dtype=np.float32), (P, 1)).astype(
        mybir.dt.np(bf16))
    iota512 = np.tile(np.arange(GOUT, dtype=np.float32), (P, 1))

    in_maps = []
    for c in range(C):
        in_maps.append({
            "xb": xb,
            "xown": np.ascontiguousarray(
                x_pad[c * NshardP:(c + 1) * NshardP]),
            "idx": idx16[c],
            "slot": slot_arr[c],
            "batchsel": batchsel[c],
            "npad": np.full((P, 1), npad[c], np.float32),
            "wpack": wpack,
            "bpack": bpack,
            "iota128": iota128,
            "iota512": iota512,
        })
    return cfg, in_maps


# ----------------------------------------------------------------------------
# Device program
# ----------------------------------------------------------------------------

def build_device(tc, io, cfg):
    nc = tc.nc
    C, B, Th, Gb = cfg.C, cfg.B, cfg.Th, cfg.Gb
    rg = [list(range(C))]
    AF = mybir.ActivationFunctionType
    OP = mybir.AluOpType
    no_cc = cfg.no_cc

    # internal DRAM ("Shared" outputs only supported for >4-core groups)
    sh = "Shared" if C > 4 else "Local"
    h_dram = nc.dram_tensor("h_rep", [cfg.Np, F], bf16, kind="Internal",
                            addr_space=sh)
    vsh_dram = nc.dram_tensor("v_shard", [cfg.Nshard, F], bf16, kind="Internal")
    st_in = nc.dram_tensor("st_in", [P, 2], f32, kind="Internal")
    st_out = nc.dram_tensor("st_out", [P, 2], f32, kind="Internal",
                            addr_space=sh)
    pl_in = nc.dram_tensor("pl_in", [P, GOUT], f32, kind="Internal")
    pl_out = nc.dram_tensor("pl_out", [P, GOUT], f32, kind="Internal",
                            addr_space=sh)

    ctx = ExitStack()
    cpool = ctx.enter_context(tc.tile_pool(name="consts", bufs=1))
    gpool = ctx.enter_context(tc.tile_pool(name="gather", bufs=2))
    opool = ctx.enter_context(tc.tile_pool(name="onehot", bufs=4))
    zpool = ctx.enter_context(tc.tile_pool(name="work", bufs=3))
    spool = ctx.enter_context(tc.tile_pool(name="small", bufs=3))
    ps_agg_pool = ctx.enter_context(tc.tile_pool(name="ps_agg", bufs=2, space="PSUM"))
    ps_mlp_pool = ctx.enter_context(tc.tile_pool(name="ps_mlp", bufs=2, space="PSUM"))
    ps_tr_pool = ctx.enter_context(tc.tile_pool(name="ps_tr", bufs=2, space="PSUM"))
    ps_pool512 = ctx.enter_context(tc.tile_pool(name="ps_p512", bufs=1, space="PSUM"))

    # resident constants / state
    idx_sb = cpool.tile([P, cfg.total_tiles * 8], i16, tag="idx")
    slot_sb = cpool.tile([P, cfg.total_tiles], f32, tag="slot")
    bsel_sb = cpool.tile([P, B], f32, tag="bsel")
    npad_sb = cpool.tile([P, 1], f32, tag="npad")
    w_sb = cpool.tile([P, 10 * F], f32, tag="w")
    bb_sb = cpool.tile([P, 20], f32, tag="bb")
    io128_sb = cpool.tile([P, P], bf16, tag="io128")
    io512_sb = cpool.tile([P, GOUT], f32, tag="io512")
    vT_sb = cpool.tile([P, B * F], f32, tag="vT")
    hTown_sb = cpool.tile([P, B * F], f32, tag="hTown")
    ssum_sb = cpool.tile([P, B], f32, tag="ssum")
    ssq_sb = cpool.tile([P, B], f32, tag="ssq")
    ident_sb = cpool.tile([P, P], f32, tag="ident")
    hpad_sb = cpool.tile([P, 1], f32, tag="hpad")

    nc.sync.dma_start(out=idx_sb[:], in_=io["idx"][:])
    nc.sync.dma_start(out=slot_sb[:], in_=io["slot"][:])
    nc.sync.dma_start(out=bsel_sb[:], in_=io["batchsel"][:])
    nc.sync.dma_start(out=npad_sb[:], in_=io["npad"][:])
    nc.sync.dma_start(out=w_sb[:], in_=io["wpack"][:])
    nc.sync.dma_start(out=bb_sb[:], in_=io["bpack"][:])
    nc.sync.dma_start(out=io128_sb[:], in_=io["iota128"][:])
    nc.sync.dma_start(out=io512_sb[:], in_=io["iota512"][:])
    make_identity(nc, ident_sb[:])
    nc.vector.memset(hpad_sb[:], 0.0)
    from concourse import library_config
    nc.gpsimd.load_library(library_config.mlp)

    # hTown <- x_own^T (fp32 self-term, transposed layout)
    for b in range(B):
        xo = zpool.tile([P, F], f32, tag="xo")
        nc.sync.dma_start(out=xo[:], in_=io["xown"][b * P:(b + 1) * P, :])
        ps_x = ps_tr_pool.tile([P, F], f32, tag="tr")
        nc.tensor.transpose(ps_x[:], xo[:], ident_sb[:])
        nc.any.tensor_copy(out=hTown_sb[:, b * F:(b + 1) * F], in_=ps_x[:])

    for l in cfg.layers:
        w1 = w_sb[:, l * 2 * F:(l * 2 + 1) * F]
        w2 = w_sb[:, (l * 2 + 1) * F:(l * 2 + 2) * F]
        b1 = bb_sb[:, 4 * l + 0:4 * l + 1]
        b2 = bb_sb[:, 4 * l + 1:4 * l + 2]
        ga = bb_sb[:, 4 * l + 2:4 * l + 3]
        be = bb_sb[:, 4 * l + 3:4 * l + 4]
        H = cfg.H
        if l == 0:
            src_lo, src_hi = io["xb"][0:H, :], io["xb"][H:cfg.Np, :]
        else:
            src_lo, src_hi = h_dram.ap()[0:H, :], h_dram.ap()[H:cfg.Np, :]

        # ---- gather + aggregate + MLP, blockwise --------------------------
        for g in range(cfg.n_groups):
            b_lo = g * Gb
            nb = cfg.nb_of_group[g]
            ntc = nb * Th                       # tiles per half-call
            tb0 = cfg.tile_base[g]
            gbuf = gpool.tile([P, 2 * ntc * F], bf16, tag="gbuf")
            ni = ntc * P
            if cfg.skip_agg < 2:
                for hf, src_h in ((0, src_lo), (1, src_hi)):
                    c0 = (tb0 + hf * ntc) * 8
                    nc.gpsimd.dma_gather(
                        gbuf[:, hf * ntc * F:(hf + 1) * ntc * F].rearrange(
                            "p (k d) -> p k d", d=F),
                        src_h,
                        idx_sb[:, c0:c0 + ntc * 8],
                        ni, ni, F, single_packet=False,
                        queue_num=(2 * g + hf) % 4)
            for bi in range(nb):
                b = b_lo + bi
                if cfg.skip_agg == 0:
                    ps_agg = ps_agg_pool.tile([P, F], f32, tag="agg")
                    for t in range(2 * Th):
                        hf, th = (0, t) if t < Th else (1, t - Th)
                        ltile = hf * ntc + bi * Th + th
                        oh = opool.tile([P, F], bf16, tag="oh")
                        col = tb0 + ltile
                        nc.any.tensor_scalar(
                            out=oh[:], in0=io128_sb[:],
                            scalar1=slot_sb[:, col:col + 1], scalar2=None,
                            op0=OP.is_equal)
                        nc.tensor.matmul(
                            ps_agg[:],
                            lhsT=gbuf[:, ltile * F:(ltile + 1) * F],
                            rhs=oh[:],
                            start=(t == 0), stop=(t == 2 * Th - 1))
                zT = zpool.tile([P, F], f32, tag="zT")
                if cfg.skip_agg == 0:
                    nc.any.tensor_tensor(
                        out=zT[:], in0=ps_agg[:],
                        in1=hTown_sb[:, b * F:(b + 1) * F], op=OP.add)
                else:
                    nc.any.tensor_tensor(
                        out=zT[:], in0=hTown_sb[:, b * F:(b + 1) * F],
                        in1=hTown_sb[:, b * F:(b + 1) * F], op=OP.add)
                ps_u = ps_mlp_pool.tile([P, F], f32, tag="mlp")
                nc.tensor.matmul(ps_u[:], lhsT=w1, rhs=zT[:], start=True, stop=True)
                uT = zpool.tile([P, F], f32, tag="uT")
                nc.scalar.activation(uT[:], ps_u[:], AF.Relu, bias=b1, scale=1.0)
                ps_v = ps_mlp_pool.tile([P, F], f32, tag="mlp")
                nc.tensor.matmul(ps_v[:], lhsT=w2, rhs=uT[:], start=True, stop=True)
                vT = vT_sb[:, b * F:(b + 1) * F]
                nc.any.tensor_scalar(
                    out=vT, in0=ps_v[:], scalar1=b2, scalar2=None, op0=OP.add,
                    op1=OP.add, accum_out=ssum_sb[:, b:b + 1])
                sq = zpool.tile([P, F], f32, tag="sq")
                nc.scalar.activation(sq[:], vT, AF.Square,
                                     accum_out=ssq_sb[:, b:b + 1])

        # ---- BN statistics (+ padding-node correction) --------------------
        s1 = spool.tile([P, 1], f32, tag="s1")
        s2 = spool.tile([P, 1], f32, tag="s2")
        nc.vector.reduce_sum(s1[:], ssum_sb[:], axis=mybir.AxisListType.X)
        nc.vector.reduce_sum(s2[:], ssq_sb[:], axis=mybir.AxisListType.X)
        # v_pad = W2.T@relu(W1.T@hpad + b1) + b2  (value of every pad node)
        ps_zp = ps_tr_pool.tile([P, 1], f32, tag="tr")
        nc.tensor.matmul(ps_zp[:], lhsT=w1, rhs=hpad_sb[:], start=True, stop=True)
        upad = spool.tile([P, 1], f32, tag="upad")
        nc.scalar.activation(upad[:], ps_zp[:], AF.Relu, bias=b1, scale=1.0)
        ps_vp = ps_tr_pool.tile([P, 1], f32, tag="tr")
        nc.tensor.matmul(ps_vp[:], lhsT=w2, rhs=upad[:], start=True, stop=True)
        vpad = spool.tile([P, 1], f32, tag="vpad")
        nc.any.tensor_scalar(out=vpad[:], in0=ps_vp[:], scalar1=b2,
                             scalar2=None, op0=OP.add)
        t1 = spool.tile([P, 1], f32, tag="t1")
        nc.any.tensor_tensor(out=t1[:], in0=vpad[:], in1=npad_sb[:], op=OP.mult)
        nc.any.tensor_tensor(out=s1[:], in0=s1[:], in1=t1[:], op=OP.subtract)
        vpad2 = spool.tile([P, 1], f32, tag="vpad2")
        nc.scalar.activation(vpad2[:], vpad[:], AF.Square)
        t2 = spool.tile([P, 1], f32, tag="t2")
        nc.any.tensor_tensor(out=t2[:], in0=vpad2[:], in1=npad_sb[:], op=OP.mult)
        nc.any.tensor_tensor(out=s2[:], in0=s2[:], in1=t2[:], op=OP.subtract)

        stt = spool.tile([P, 2], f32, tag="stt")
        nc.any.tensor_copy(out=stt[:, 0:1], in_=s1[:])
        nc.any.tensor_copy(out=stt[:, 1:2], in_=s2[:])
        nc.sync.dma_start(out=st_in.ap(), in_=stt[:])
        if not no_cc:
            nc.gpsimd.collective_compute(
                "AllReduce", OP.add, replica_groups=rg,
                ins=[st_in.ap()], outs=[st_out.ap()])
            st_res = st_out
        else:
            st_res = st_in
        stg = spool.tile([P, 2], f32, tag="stg")
        nc.sync.dma_start(out=stg[:], in_=st_res.ap())

        inv_n = 1.0 / float(cfg.N)
        mu = spool.tile([P, 1], f32, tag="mu")
        nc.any.tensor_scalar(out=mu[:], in0=stg[:, 0:1], scalar1=inv_n,
                             scalar2=None, op0=OP.mult)
        ms = spool.tile([P, 1], f32, tag="ms")
        nc.any.tensor_scalar(out=ms[:], in0=stg[:, 1:2], scalar1=inv_n,
                             scalar2=None, op0=OP.mult)
        mu2 = spool.tile([P, 1], f32, tag="mu2")
        nc.scalar.activation(mu2[:], mu[:], AF.Square)
        var = spool.tile([P, 1], f32, tag="var")
        nc.any.tensor_tensor(out=var[:], in0=ms[:], in1=mu2[:], op=OP.subtract)
        veps = spool.tile([P, 1], f32, tag="veps")
        nc.any.tensor_scalar(out=veps[:], in0=var[:], scalar1=BN_EPS,
                             scalar2=None, op0=OP.add)
        sd = spool.tile([P, 1], f32, tag="sd")
        nc.scalar.activation(sd[:], veps[:], AF.Sqrt)
        rs = spool.tile([P, 1], f32, tag="rs")
        nc.vector.reciprocal(rs[:], sd[:])
        aa = spool.tile([P, 1], f32, tag="aa")
        nc.any.tensor_tensor(out=aa[:], in0=rs[:], in1=ga, op=OP.mult)
        mua = spool.tile([P, 1], f32, tag="mua")
        nc.any.tensor_tensor(out=mua[:], in0=mu[:], in1=aa[:], op=OP.mult)
        cc = spool.tile([P, 1], f32, tag="cc")
        nc.any.tensor_tensor(out=cc[:], in0=be, in1=mua[:], op=OP.subtract)

        # hpad' = relu?(a * v_pad + c)
        hp1 = spool.tile([P, 1], f32, tag="hp1")
        nc.any.tensor_tensor(out=hp1[:], in0=vpad[:], in1=aa[:], op=OP.mult)
        if RELU_AFTER[l]:
            hp2 = spool.tile([P, 1], f32, tag="hp2")
            nc.any.tensor_tensor(out=hp2[:], in0=hp1[:], in1=cc[:], op=OP.add)
            nc.scalar.activation(hpad_sb[:], hp2[:], AF.Relu)
        else:
            nc.any.tensor_tensor(out=hpad_sb[:], in0=hp1[:], in1=cc[:],
                                 op=OP.add)

        # ---- normalize (into hTown) + transpose + (AllGather | pool) ------
        if l == 4:
            ps_pool = ps_pool512.tile([P, GOUT], f32, tag="p512")
        for b in range(B):
            nt = hTown_sb[:, b * F:(b + 1) * F]
            if RELU_AFTER[l]:
                nc.scalar.activation(nt, vT_sb[:, b * F:(b + 1) * F],
                                     AF.Relu, bias=cc[:], scale=aa[:])
            else:
                nc.any.tensor_scalar(
                    out=nt, in0=vT_sb[:, b * F:(b + 1) * F],
                    scalar1=aa[:], scalar2=cc[:], op0=OP.mult, op1=OP.add)
            ps_t = ps_tr_pool.tile([P, F], f32, tag="tr")
            nc.tensor.transpose(ps_t[:], nt, ident_sb[:])
            if l < 4:
                ntr = zpool.tile([P, F], bf16, tag="ntr")
                nc.any.tensor_copy(out=ntr[:], in_=ps_t[:])
                nc.sync.dma_start(out=vsh_dram.ap()[b * P:(b + 1) * P, :],
                                  in_=ntr[:])
            else:
                ntr = zpool.tile([P, F], f32, tag="ntrf")
                nc.any.tensor_copy(out=ntr[:], in_=ps_t[:])
                poh = opool.tile([P, GOUT], f32, tag="poh")
                nc.any.tensor_scalar(
                    out=poh[:], in0=io512_sb[:],
                    scalar1=bsel_sb[:, b:b + 1], scalar2=None, op0=OP.is_equal)
                nc.tensor.matmul(ps_pool[:], lhsT=ntr[:], rhs=poh[:],
                                 start=(b == 0), stop=(b == B - 1))
        if l < 4:
            if not no_cc:
                nc.gpsimd.collective_compute(
                    "AllGather", OP.bypass, replica_groups=rg,
                    ins=[vsh_dram.ap()], outs=[h_dram.ap()])
            else:
                nc.sync.dma_start(out=h_dram.ap()[0:cfg.Nshard, :],
                                  in_=vsh_dram.ap())

    # ---- pooled AllReduce + transpose out ---------------------------------
    pooledT = cpool.tile([P, GOUT], f32, tag="pooledT")
    nc.any.tensor_copy(out=pooledT[:], in_=ps_pool[:])
    nc.sync.dma_start(out=pl_in.ap(), in_=pooledT[:])
    if not no_cc:
        nc.gpsimd.collective_compute(
            "AllReduce", OP.add, replica_groups=rg,
            ins=[pl_in.ap()], outs=[pl_out.ap()])
        pl_res = pl_out
    else:
        pl_res = pl_in
    plr = cpool.tile([P, GOUT], f32, tag="plr")
    nc.sync.dma_start(out=plr[:], in_=pl_res.ap())
    for q in range(GOUT // P):
        ps_q = ps_tr_pool.tile([P, P], f32, tag="tr")
        nc.tensor.transpose(ps_q[:], plr[:, q * P:(q + 1) * P], ident_sb[:])
        oq = zpool.tile([P, P], f32, tag="oq")
        nc.any.tensor_copy(out=oq[:], in_=ps_q[:])
        nc.sync.dma_start(out=io["zg"][q * P:(q + 1) * P, :], in_=oq[:])
    ctx.close()


# ----------------------------------------------------------------------------
# Entry point
# ----------------------------------------------------------------------------

_CACHE = {}
_LAST_RESULTS = None


def _build_full(cfg):
    nc = bacc.Bacc("TRN2", target_bir_lowering=False, debug=False,
                   num_devices=cfg.C, num_swdge_queues=4)
    io = {}
    io["xb"] = nc.dram_tensor("xb", [cfg.Np, F], bf16,
                              kind="ExternalInput").ap()
    io["xown"] = nc.dram_tensor("xown", [cfg.Nshard, F], f32,
                                kind="ExternalInput").ap()
    io["idx"] = nc.dram_tensor("idx", [P, cfg.total_tiles * 8], i16,
                               kind="ExternalInput").ap()
    io["slot"] = nc.dram_tensor("slot", [P, cfg.total_tiles], f32,
                                kind="ExternalInput").ap()
    io["batchsel"] = nc.dram_tensor("batchsel", [P, cfg.B], f32,
                                    kind="ExternalInput").ap()
    io["npad"] = nc.dram_tensor("npad", [P, 1], f32, kind="ExternalInput").ap()
    io["wpack"] = nc.dram_tensor("wpack", [P, 10 * F], f32,
                                 kind="ExternalInput").ap()
    io["bpack"] = nc.dram_tensor("bpack", [P, 20], f32,
                                 kind="ExternalInput").ap()
    io["iota128"] = nc.dram_tensor("iota128", [P, P], bf16,
                                   kind="ExternalInput").ap()
    io["iota512"] = nc.dram_tensor("iota512", [P, GOUT], f32,
                                   kind="ExternalInput").ap()
    io["zg"] = nc.dram_tensor("zg", [GOUT, F], f32, kind="ExternalOutput").ap()
    with tile.TileContext(nc) as tc:
        build_device(tc, io, cfg)
    nc.compile()
    return nc


def kernel(**inputs):
    import os
    global _LAST_RESULTS
    x = np.asarray(inputs["x"], np.float32)
    edge_index = np.asarray(inputs["edge_index"])
    batch = np.asarray(inputs["batch"])
    C = 8
    cfg, in_maps = prep_host(x, edge_index, batch, inputs, C=C)

    key = (x.shape, edge_index.shape, cfg.Th)
    if key not in _CACHE:
        _CACHE[key] = _build_full(cfg)
    nc = _CACHE[key]

    trace = bool(os.environ.get("GNN_TRACE"))
    tmpdir = os.environ.get("GNN_TRACE_DIR") or None
    res = run_bass_kernel_spmd(nc, in_maps, core_ids=list(range(C)),
                               trace=trace, tmpdir=tmpdir)
    _LAST_RESULTS = res
    zg = res.results[0]["zg"]                    # [512, 128]
    return zg.reshape(GOUT, 8, 16).astype(np.float32)



# revision 7
# speedup vs baseline: 4.5702x; 4.5702x over previous
"""Trainium2 Bass kernel for the DGCL GNN (3 GIN conv layers + 8-factor
disentangled head + global add pool).

Self-contained: host-side numpy preprocessing (graph partitioning /
weight packing), an SPMD Bass/Tile device program for 8 NeuronCores, and
the gather/unshard glue.

Structure of the computation (mathematically identical to the reference):
  - The K=8 disentangled head factors share the same edge aggregation, and
    their per-factor MLPs concatenate into [128,128] dense / block-diagonal
    matmuls.  So the network is 5 uniform layers:
        z = h + scatter_add(gather(h, src), dst)
        v = relu(z @ W1 + b1) @ W2 + b2
        h' = BN(v) (+ relu for layers 0,1,3)
    followed by a per-graph add-pool.
  - Nodes (and their incoming edges) are sharded contiguously across the 8
    cores.  Edge gathers read a bf16 replica of h from local DRAM via
    dma_gather (int16 indices -> lo/hi half split); aggregation happens as
    bf16 one-hot matmuls accumulating in fp32 PSUM.  The self term is added
    in fp32 from an SBUF-resident transposed copy of the core's own shard.
    The h replica is refreshed each layer with an AllGather; BN statistics
    and the pooled output use AllReduce.
"""

import math
from contextlib import ExitStack

import numpy as np

import concourse.bacc as bacc
import concourse.bass as bass
import concourse.mybir as mybir
import concourse.tile as tile
from concourse.bass_utils import run_bass_kernel_spmd
from concourse.masks import make_identity

P = 128
F = 128
GOUT = 512          # output graph rows (harness G = 512)
BN_EPS = 1e-5
RELU_AFTER = [True, True, False, True, False]
f32 = mybir.dt.float32
bf16 = mybir.dt.bfloat16
i16 = mybir.dt.int16


class Cfg:
    def __init__(self, N, C, Th, group_blocks=5):
        self.N = N                      # real node count
        self.C = C                      # cores
        self.Nshard = -(-N // (C * P)) * P
        self.Np = self.Nshard * C
        self.B = self.Nshard // P       # dst blocks per core
        self.Th = Th                    # 128-edge tiles per block-half
        self.Gb = group_blocks          # blocks per gather-call group
        self.n_groups = -(-self.B // group_blocks)
        self.H = self.Np // 2
        self.total_tiles = self.B * 2 * Th
        self.tile_base = None           # filled by prep_host / fill_groups
        self.nb_of_group = None
        self.no_cc = False
        self.layers = (0, 1, 2, 3, 4)   # which layer bodies to emit
        self.skip_agg = 0               # 1: no onehot/agg-mm; 2: no gathers too

    def fill_groups(self):
        tb, bases, nbs = 0, [], []
        for g in range(self.n_groups):
            nb = min((g + 1) * self.Gb, self.B) - g * self.Gb
            bases.append(tb)
            nbs.append(nb)
            tb += 2 * nb * self.Th
        self.tile_base, self.nb_of_group = bases, nbs
        return self


# ----------------------------------------------------------------------------
# Host-side preprocessing
# ----------------------------------------------------------------------------

def _blockdiag(W):
    K_, d_, _ = W.shape
    out = np.zeros((K_ * d_, K_ * d_), np.float32)
    for k in range(K_):
        out[k * d_:(k + 1) * d_, k * d_:(k + 1) * d_] = W[k]
    return out


def prep_host(x, edge_index, batch, params, C=8, group_blocks=5):
    """Returns (cfg, in_maps) ready for the device program."""
    x = np.asarray(x, np.float32)
    N = x.shape[0]
    src = np.asarray(edge_index[0], np.int64)
    dst = np.asarray(edge_index[1], np.int64)
    batch = np.asarray(batch, np.int64)

    order = np.argsort(dst, kind="stable")
    s_sorted = src[order]
    d_sorted = dst[order]

    gblock = d_sorted // P                      # global dst block id
    slot = (d_sorted % P).astype(np.float32)

    NshardP = -(-N // (C * P)) * P
    B = NshardP // P
    Np = NshardP * C
    H = Np // 2
    assert H <= 32767 and Np - H <= 32767, "half-table exceeds int16 range"

    # split each block's edge list by source half (int16 index limit)
    is_hi = (s_sorted >= H).astype(np.int64)
    halfkey = gblock * 2 + is_hi
    counts2 = np.bincount(halfkey, minlength=C * B * 2)
    Th = max(1, int(math.ceil(counts2.max() / P)))
    cfg = Cfg(N, C, Th, group_blocks).fill_groups()
    Gb = group_blocks

    total_tiles = cfg.total_tiles
    idx16 = np.zeros((C, 16, total_tiles * 8), np.int16)
    slot_arr = np.full((C, 128, total_tiles), 999.0, np.float32)

    order2 = np.argsort(halfkey, kind="stable")
    s2 = s_sorted[order2]
    slot2 = slot[order2]
    startpos = np.zeros(C * B * 2 + 1, np.int64)
    startpos[1:] = np.cumsum(counts2)

    for c in range(C):
        for b in range(B):
            g = b // Gb
            bi = b - g * Gb
            nb = cfg.nb_of_group[g]
            for hf in range(2):
                k = (c * B + b) * 2 + hf
                cnt = counts2[k]
                if cnt == 0:
                    continue
                sl = slice(startpos[k], startpos[k + 1])
                j = np.arange(cnt)
                call_tile0 = cfg.tile_base[g] + (nb * Th if hf else 0)
                gt = call_tile0 + bi * Th + j // P
                slot_arr[c, j % P, gt] = slot2[sl]
                vals = s2[sl] - (H if hf else 0)
                gcol = (call_tile0 + bi * Th) * 8 + j // 16
                idx16[c, j % 16, gcol] = vals.astype(np.int16)
    idx16 = np.tile(idx16, (1, 8, 1))           # replicate 16-row wrap x8

    # per-node batch selector (999 sentinel for padding nodes)
    ids = np.arange(Np)
    bsel = np.where(ids < N, batch[np.minimum(ids, N - 1)], 999).astype(np.float32)
    batchsel = bsel.reshape(C, B, P).transpose(0, 2, 1).copy()   # [C, P, B]

    npad = np.array(
        [max(0, (c + 1) * NshardP - max(N, c * NshardP)) for c in range(C)],
        np.float32,
    )

    # padded node features (zeros for pad rows) + bf16 gather replica
    x_pad = np.zeros((Np, F), np.float32)
    x_pad[:N] = x
    xb = x_pad.astype(mybir.dt.np(bf16))

    # --- weights ----------------------------------------------------------
    gc_W1 = np.asarray(params["gc_W1"], np.float32)
    gc_W2 = np.asarray(params["gc_W2"], np.float32)
    gc_b1 = np.asarray(params["gc_b1"], np.float32)
    gc_b2 = np.asarray(params["gc_b2"], np.float32)
    gc_g = np.asarray(params["gc_g"], np.float32)
    gc_be = np.asarray(params["gc_be"], np.float32)
    h0_W1 = np.asarray(params["h0_W1"], np.float32)
    h0_W2 = np.asarray(params["h0_W2"], np.float32)
    h1_W1 = np.asarray(params["h1_W1"], np.float32)
    h1_W2 = np.asarray(params["h1_W2"], np.float32)

    W1s = [gc_W1[0], gc_W1[1], gc_W1[2],
           h0_W1.transpose(1, 0, 2).reshape(F, F), _blockdiag(h1_W1)]
    W2s = [gc_W2[0], gc_W2[1], gc_W2[2], _blockdiag(h0_W2), _blockdiag(h1_W2)]
    b1s = [gc_b1[0], gc_b1[1], gc_b1[2],
           np.asarray(params["h0_b1"], np.float32).reshape(-1),
           np.asarray(params["h1_b1"], np.float32).reshape(-1)]
    b2s = [gc_b2[0], gc_b2[1], gc_b2[2],
           np.asarray(params["h0_b2"], np.float32).reshape(-1),
           np.asarray(params["h1_b2"], np.float32).reshape(-1)]
    gs = [gc_g[0], gc_g[1], gc_g[2],
          np.asarray(params["h0_g"], np.float32).reshape(-1),
          np.asarray(params["h1_g"], np.float32).reshape(-1)]
    bes = [gc_be[0], gc_be[1], gc_be[2],
           np.asarray(params["h0_be"], np.float32).reshape(-1),
           np.asarray(params["h1_be"], np.float32).reshape(-1)]

    wpack = np.concatenate(
        sum(([W1s[l], W2s[l]] for l in range(5)), []), axis=1
    ).astype(np.float32)                                        # [F, 1280]
    bpack = np.stack(
        sum(([b1s[l], b2s[l], gs[l], bes[l]] for l in range(5)), []), axis=1
    ).astype(np.float32)                                        # [F, 20]

    iota128 = np.tile(np.arange(P, dtype=np.float32), (P, 1)).astype(
        mybir.dt.np(bf16))
    iota512 = np.tile(np.arange(GOUT, dtype=np.float32), (P, 1))

    # host-precomputed one-hot tiles (graph-static, shared by all layers):
    # oh[p, t*P + s] = 1 iff slot_arr[p, t] == s  (999 pad rows stay zero)
    ohs = []
    for c in range(C):
        sl = slot_arr[c].astype(np.int32)               # [P, total_tiles]
        oh = np.zeros((P, total_tiles, P), np.float32)
        pp, tt = np.nonzero(sl < P)
        oh[pp, tt, sl[pp, tt]] = 1.0
        ohs.append(np.ascontiguousarray(
            oh.reshape(P, total_tiles * P)).astype(mybir.dt.np(bf16)))

    in_maps = []
    for c in range(C):
        in_maps.append({
            "xb": xb,
            "xown": np.ascontiguousarray(
                x_pad[c * NshardP:(c + 1) * NshardP]),
            "idx": idx16[c],
            "slot": slot_arr[c],
            "batchsel": batchsel[c],
            "npad": np.full((P, 1), npad[c], np.float32),
            "wpack": wpack,
            "bpack": bpack,
            "iota128": iota128,
            "iota512": iota512,
            "oh": ohs[c],
        })
    return cfg, in_maps


# ----------------------------------------------------------------------------
# Device program
# ----------------------------------------------------------------------------

def build_device(tc, io, cfg):
    nc = tc.nc
    C, B, Th, Gb = cfg.C, cfg.B, cfg.Th, cfg.Gb
    rg = [list(range(C))]
    AF = mybir.ActivationFunctionType
    OP = mybir.AluOpType
    no_cc = cfg.no_cc

    # internal DRAM ("Shared" outputs only supported for >4-core groups)
    sh = "Shared" if C > 4 else "Local"
    h_dram = nc.dram_tensor("h_rep", [cfg.Np, F], bf16, kind="Internal",
                            addr_space=sh)
    vsh_dram = nc.dram_tensor("v_shard", [cfg.Nshard, F], bf16, kind="Internal")
    st_in = nc.dram_tensor("st_in", [P, 2], f32, kind="Internal")
    st_out = nc.dram_tensor("st_out", [P, 2], f32, kind="Internal",
                            addr_space=sh)
    pl_in = nc.dram_tensor("pl_in", [P, GOUT], f32, kind="Internal")
    pl_out = nc.dram_tensor("pl_out", [P, GOUT], f32, kind="Internal",
                            addr_space=sh)

    ctx = ExitStack()
    cpool = ctx.enter_context(tc.tile_pool(name="consts", bufs=1))
    gpool = ctx.enter_context(tc.tile_pool(name="gather", bufs=2))
    opool = ctx.enter_context(tc.tile_pool(name="onehot", bufs=4))
    zpool = ctx.enter_context(tc.tile_pool(name="work", bufs=3))
    spool = ctx.enter_context(tc.tile_pool(name="small", bufs=3))
    ps_agg_pool = ctx.enter_context(tc.tile_pool(name="ps_agg", bufs=2, space="PSUM"))
    ps_mlp_pool = ctx.enter_context(tc.tile_pool(name="ps_mlp", bufs=2, space="PSUM"))
    ps_tr_pool = ctx.enter_context(tc.tile_pool(name="ps_tr", bufs=2, space="PSUM"))
    ps_pool512 = ctx.enter_context(tc.tile_pool(name="ps_p512", bufs=1, space="PSUM"))

    # resident constants / state
    idx_sb = cpool.tile([P, cfg.total_tiles * 8], i16, tag="idx")
    bsel_sb = cpool.tile([P, B], f32, tag="bsel")
    npad_sb = cpool.tile([P, 1], f32, tag="npad")
    w_sb = cpool.tile([P, 10 * F], f32, tag="w")
    bb_sb = cpool.tile([P, 20], f32, tag="bb")
    io128_sb = cpool.tile([P, P], bf16, tag="io128")
    io512_sb = cpool.tile([P, GOUT], f32, tag="io512")
    vT_sb = cpool.tile([P, B * F], f32, tag="vT")
    hTown_sb = cpool.tile([P, B * F], f32, tag="hTown")
    ssum_sb = cpool.tile([P, B], f32, tag="ssum")
    ssq_sb = cpool.tile([P, B], f32, tag="ssq")
    ident_sb = cpool.tile([P, P], f32, tag="ident")
    hpad_sb = cpool.tile([P, 1], f32, tag="hpad")

    nc.sync.dma_start(out=idx_sb[:], in_=io["idx"][:])
    nc.sync.dma_start(out=bsel_sb[:], in_=io["batchsel"][:])
    nc.sync.dma_start(out=npad_sb[:], in_=io["npad"][:])
    nc.sync.dma_start(out=w_sb[:], in_=io["wpack"][:])
    nc.sync.dma_start(out=bb_sb[:], in_=io["bpack"][:])
    nc.sync.dma_start(out=io128_sb[:], in_=io["iota128"][:])
    nc.sync.dma_start(out=io512_sb[:], in_=io["iota512"][:])
    make_identity(nc, ident_sb[:])
    nc.vector.memset(hpad_sb[:], 0.0)
    from concourse import library_config
    nc.gpsimd.load_library(library_config.mlp)

    # hTown <- x_own^T (fp32 self-term, transposed layout)
    for b in range(B):
        xo = zpool.tile([P, F], f32, tag="xo")
        nc.sync.dma_start(out=xo[:], in_=io["xown"][b * P:(b + 1) * P, :])
        ps_x = ps_tr_pool.tile([P, F], f32, tag="tr")
        nc.tensor.transpose(ps_x[:], xo[:], ident_sb[:])
        nc.any.tensor_copy(out=hTown_sb[:, b * F:(b + 1) * F], in_=ps_x[:])

    for l in cfg.layers:
        w1 = w_sb[:, l * 2 * F:(l * 2 + 1) * F]
        w2 = w_sb[:, (l * 2 + 1) * F:(l * 2 + 2) * F]
        b1 = bb_sb[:, 4 * l + 0:4 * l + 1]
        b2 = bb_sb[:, 4 * l + 1:4 * l + 2]
        ga = bb_sb[:, 4 * l + 2:4 * l + 3]
        be = bb_sb[:, 4 * l + 3:4 * l + 4]
        H = cfg.H
        if l == 0:
            src_lo, src_hi = io["xb"][0:H, :], io["xb"][H:cfg.Np, :]
        else:
            src_lo, src_hi = h_dram.ap()[0:H, :], h_dram.ap()[H:cfg.Np, :]

        # ---- gather + aggregate + MLP, blockwise --------------------------
        for g in range(cfg.n_groups):
            b_lo = g * Gb
            nb = cfg.nb_of_group[g]
            ntc = nb * Th                       # tiles per half-call
            tb0 = cfg.tile_base[g]
            gbuf = gpool.tile([P, 2 * ntc * F], bf16, tag="gbuf")
            ohb = gpool.tile([P, 2 * ntc * P], bf16, tag="ohbuf")
            nc.sync.dma_start(
                out=ohb[:], in_=io["oh"][:, tb0 * P:(tb0 + 2 * ntc) * P])
            ni = ntc * P
            if cfg.skip_agg < 2:
                for hf, src_h in ((0, src_lo), (1, src_hi)):
                    c0 = (tb0 + hf * ntc) * 8
                    nc.gpsimd.dma_gather(
                        gbuf[:, hf * ntc * F:(hf + 1) * ntc * F].rearrange(
                            "p (k d) -> p k d", d=F),
                        src_h,
                        idx_sb[:, c0:c0 + ntc * 8],
                        ni, ni, F, single_packet=False,
                        queue_num=(2 * g + hf) % 4)
            for bi in range(nb):
                b = b_lo + bi
                if cfg.skip_agg == 0:
                    ps_agg = ps_agg_pool.tile([P, F], f32, tag="agg")
                    for t in range(2 * Th):
                        hf, th = (0, t) if t < Th else (1, t - Th)
                        ltile = hf * ntc + bi * Th + th
                        nc.tensor.matmul(
                            ps_agg[:],
                            lhsT=gbuf[:, ltile * F:(ltile + 1) * F],
                            rhs=ohb[:, ltile * P:(ltile + 1) * P],
                            start=(t == 0), stop=(t == 2 * Th - 1))
                zT = zpool.tile([P, F], f32, tag="zT")
                if cfg.skip_agg == 0:
                    nc.any.tensor_tensor(
                        out=zT[:], in0=ps_agg[:],
                        in1=hTown_sb[:, b * F:(b + 1) * F], op=OP.add)
                else:
                    nc.any.tensor_tensor(
                        out=zT[:], in0=hTown_sb[:, b * F:(b + 1) * F],
                        in1=hTown_sb[:, b * F:(b + 1) * F], op=OP.add)
                ps_u = ps_mlp_pool.tile([P, F], f32, tag="mlp")
                nc.tensor.matmul(ps_u[:], lhsT=w1, rhs=zT[:], start=True, stop=True)
                uT = zpool.tile([P, F], f32, tag="uT")
                nc.scalar.activation(uT[:], ps_u[:], AF.Relu, bias=b1, scale=1.0)
                ps_v = ps_mlp_pool.tile([P, F], f32, tag="mlp")
                nc.tensor.matmul(ps_v[:], lhsT=w2, rhs=uT[:], start=True, stop=True)
                vT = vT_sb[:, b * F:(b + 1) * F]
                nc.any.tensor_scalar(
                    out=vT, in0=ps_v[:], scalar1=b2, scalar2=None, op0=OP.add,
                    op1=OP.add, accum_out=ssum_sb[:, b:b + 1])
                sq = zpool.tile([P, F], f32, tag="sq")
                nc.scalar.activation(sq[:], vT, AF.Square,
                                     accum_out=ssq_sb[:, b:b + 1])

        # ---- BN statistics (+ padding-node correction) --------------------
        s1 = spool.tile([P, 1], f32, tag="s1")
        s2 = spool.tile([P, 1], f32, tag="s2")
        nc.vector.reduce_sum(s1[:], ssum_sb[:], axis=mybir.AxisListType.X)
        nc.vector.reduce_sum(s2[:], ssq_sb[:], axis=mybir.AxisListType.X)
        # v_pad = W2.T@relu(W1.T@hpad + b1) + b2  (value of every pad node)
        ps_zp = ps_tr_pool.tile([P, 1], f32, tag="tr")
        nc.tensor.matmul(ps_zp[:], lhsT=w1, rhs=hpad_sb[:], start=True, stop=True)
        upad = spool.tile([P, 1], f32, tag="upad")
        nc.scalar.activation(upad[:], ps_zp[:], AF.Relu, bias=b1, scale=1.0)
        ps_vp = ps_tr_pool.tile([P, 1], f32, tag="tr")
        nc.tensor.matmul(ps_vp[:], lhsT=w2, rhs=upad[:], start=True, stop=True)
        vpad = spool.tile([P, 1], f32, tag="vpad")
        nc.any.tensor_scalar(out=vpad[:], in0=ps_vp[:], scalar1=b2,
                             scalar2=None, op0=OP.add)
        t1 = spool.tile([P, 1], f32, tag="t1")
        nc.any.tensor_tensor(out=t1[:], in0=vpad[:], in1=npad_sb[:], op=OP.mult)
        nc.any.tensor_tensor(out=s1[:], in0=s1[:], in1=t1[:], op=OP.subtract)
        vpad2 = spool.tile([P, 1], f32, tag="vpad2")
        nc.scalar.activation(vpad2[:], vpad[:], AF.Square)
        t2 = spool.tile([P, 1], f32, tag="t2")
        nc.any.tensor_tensor(out=t2[:], in0=vpad2[:], in1=npad_sb[:], op=OP.mult)
        nc.any.tensor_tensor(out=s2[:], in0=s2[:], in1=t2[:], op=OP.subtract)

        stt = spool.tile([P, 2], f32, tag="stt")
        nc.any.tensor_copy(out=stt[:, 0:1], in_=s1[:])
        nc.any.tensor_copy(out=stt[:, 1:2], in_=s2[:])
        nc.sync.dma_start(out=st_in.ap(), in_=stt[:])
        if not no_cc:
            nc.gpsimd.collective_compute(
                "AllReduce", OP.add, replica_groups=rg,
                ins=[st_in.ap()], outs=[st_out.ap()])
            st_res = st_out
        else:
            st_res = st_in
        stg = spool.tile([P, 2], f32, tag="stg")
        nc.sync.dma_start(out=stg[:], in_=st_res.ap())

        inv_n = 1.0 / float(cfg.N)
        mu = spool.tile([P, 1], f32, tag="mu")
        nc.any.tensor_scalar(out=mu[:], in0=stg[:, 0:1], scalar1=inv_n,
                             scalar2=None, op0=OP.mult)
        ms = spool.tile([P, 1], f32, tag="ms")
        nc.any.tensor_scalar(out=ms[:], in0=stg[:, 1:2], scalar1=inv_n,
                             scalar2=None, op0=OP.mult)
        mu2 = spool.tile([P, 1], f32, tag="mu2")
        nc.scalar.activation(mu2[:], mu[:], AF.Square)
        var = spool.tile([P, 1], f32, tag="var")
        nc.any.tensor_tensor(out=var[:], in0=ms[:], in1=mu2[:], op=OP.subtract)
        veps = spool.tile([P, 1], f32, tag="veps")
        nc.any.tensor_scalar(out=veps[:], in0=var[:], scalar1=BN_EPS,
                             scalar2=None, op0=OP.add)
        sd = spool.tile([P, 1], f32, tag="sd")
        nc.scalar.activation(sd[:], veps[:], AF.Sqrt)
        rs = spool.tile([P, 1], f32, tag="rs")
        nc.vector.reciprocal(rs[:], sd[:])
        aa = spool.tile([P, 1], f32, tag="aa")
        nc.any.tensor_tensor(out=aa[:], in0=rs[:], in1=ga, op=OP.mult)
        mua = spool.tile([P, 1], f32, tag="mua")
        nc.any.tensor_tensor(out=mua[:], in0=mu[:], in1=aa[:], op=OP.mult)
        cc = spool.tile([P, 1], f32, tag="cc")
        nc.any.tensor_tensor(out=cc[:], in0=be, in1=mua[:], op=OP.subtract)

        # hpad' = relu?(a * v_pad + c)
        hp1 = spool.tile([P, 1], f32, tag="hp1")
        nc.any.tensor_tensor(out=hp1[:], in0=vpad[:], in1=aa[:], op=OP.mult)
        if RELU_AFTER[l]:
            hp2 = spool.tile([P, 1], f32, tag="hp2")
            nc.any.tensor_tensor(out=hp2[:], in0=hp1[:], in1=cc[:], op=OP.add)
            nc.scalar.activation(hpad_sb[:], hp2[:], AF.Relu)
        else:
            nc.any.tensor_tensor(out=hpad_sb[:], in0=hp1[:], in1=cc[:],
                                 op=OP.add)

        # ---- normalize (into hTown) + transpose + (AllGather | pool) ------
        if l == 4:
            ps_pool = ps_pool512.tile([P, GOUT], f32, tag="p512")
        for b in range(B):
            nt = hTown_sb[:, b * F:(b + 1) * F]
            if RELU_AFTER[l]:
                nc.scalar.activation(nt, vT_sb[:, b * F:(b + 1) * F],
                                     AF.Relu, bias=cc[:], scale=aa[:])
            else:
                nc.any.tensor_scalar(
                    out=nt, in0=vT_sb[:, b * F:(b + 1) * F],
                    scalar1=aa[:], scalar2=cc[:], op0=OP.mult, op1=OP.add)
            ps_t = ps_tr_pool.tile([P, F], f32, tag="tr")
            nc.tensor.transpose(ps_t[:], nt, ident_sb[:])
            if l < 4:
                ntr = zpool.tile([P, F], bf16, tag="ntr")
                nc.any.tensor_copy(out=ntr[:], in_=ps_t[:])
                nc.sync.dma_start(out=vsh_dram.ap()[b * P:(b + 1) * P, :],
                                  in_=ntr[:])
            else:
                ntr = zpool.tile([P, F], f32, tag="ntrf")
                nc.any.tensor_copy(out=ntr[:], in_=ps_t[:])
                poh = opool.tile([P, GOUT], f32, tag="poh")
                nc.any.tensor_scalar(
                    out=poh[:], in0=io512_sb[:],
                    scalar1=bsel_sb[:, b:b + 1], scalar2=None, op0=OP.is_equal)
                nc.tensor.matmul(ps_pool[:], lhsT=ntr[:], rhs=poh[:],
                                 start=(b == 0), stop=(b == B - 1))
        if l < 4:
            if not no_cc:
                nc.gpsimd.collective_compute(
                    "AllGather", OP.bypass, replica_groups=rg,
                    ins=[vsh_dram.ap()], outs=[h_dram.ap()])
            else:
                nc.sync.dma_start(out=h_dram.ap()[0:cfg.Nshard, :],
                                  in_=vsh_dram.ap())

    # ---- pooled AllReduce + transpose out ---------------------------------
    pooledT = cpool.tile([P, GOUT], f32, tag="pooledT")
    nc.any.tensor_copy(out=pooledT[:], in_=ps_pool[:])
    nc.sync.dma_start(out=pl_in.ap(), in_=pooledT[:])
    if not no_cc:
        nc.gpsimd.collective_compute(
            "AllReduce", OP.add, replica_groups=rg,
            ins=[pl_in.ap()], outs=[pl_out.ap()])
        pl_res = pl_out
    else:
        pl_res = pl_in
    plr = cpool.tile([P, GOUT], f32, tag="plr")
    nc.sync.dma_start(out=plr[:], in_=pl_res.ap())
    for q in range(GOUT // P):
        ps_q = ps_tr_pool.tile([P, P], f32, tag="tr")
        nc.tensor.transpose(ps_q[:], plr[:, q * P:(q + 1) * P], ident_sb[:])
        oq = zpool.tile([P, P], f32, tag="oq")
        nc.any.tensor_copy(out=oq[:], in_=ps_q[:])
        nc.sync.dma_start(out=io["zg"][q * P:(q + 1) * P, :], in_=oq[:])
    ctx.close()


# ----------------------------------------------------------------------------
# Entry point
# ----------------------------------------------------------------------------

_CACHE = {}
_LAST_RESULTS = None


def _build_full(cfg):
    nc = bacc.Bacc("TRN2", target_bir_lowering=False, debug=False,
                   num_devices=cfg.C, num_swdge_queues=4)
    io = {}
    io["xb"] = nc.dram_tensor("xb", [cfg.Np, F], bf16,
                              kind="ExternalInput").ap()
    io["xown"] = nc.dram_tensor("xown", [cfg.Nshard, F], f32,
                                kind="ExternalInput").ap()
    io["idx"] = nc.dram_tensor("idx", [P, cfg.total_tiles * 8], i16,
                               kind="ExternalInput").ap()
    io["slot"] = nc.dram_tensor("slot", [P, cfg.total_tiles], f32,
                                kind="ExternalInput").ap()
    io["batchsel"] = nc.dram_tensor("batchsel", [P, cfg.B], f32,
                                    kind="ExternalInput").ap()
    io["npad"] = nc.dram_tensor("npad", [P, 1], f32, kind="ExternalInput").ap()
    io["wpack"] = nc.dram_tensor("wpack", [P, 10 * F], f32,
                                 kind="ExternalInput").ap()
    io["bpack"] = nc.dram_tensor("bpack", [P, 20], f32,
                                 kind="ExternalInput").ap()
    io["iota128"] = nc.dram_tensor("iota128", [P, P], bf16,
                                   kind="ExternalInput").ap()
    io["iota512"] = nc.dram_tensor("iota512", [P, GOUT], f32,
                                   kind="ExternalInput").ap()
    io["oh"] = nc.dram_tensor("oh", [P, cfg.total_tiles * P], bf16,
                              kind="ExternalInput").ap()
    io["zg"] = nc.dram_tensor("zg", [GOUT, F], f32, kind="ExternalOutput").ap()
    with tile.TileContext(nc) as tc:
        build_device(tc, io, cfg)
    nc.compile()
    return nc


def kernel(**inputs):
    import os
    global _LAST_RESULTS
    x = np.asarray(inputs["x"], np.float32)
    edge_index = np.asarray(inputs["edge_index"])
    batch = np.asarray(inputs["batch"])
    C = 8
    cfg, in_maps = prep_host(x, edge_index, batch, inputs, C=C)

    key = (x.shape, edge_index.shape, cfg.Th)
    if key not in _CACHE:
        _CACHE[key] = _build_full(cfg)
    nc = _CACHE[key]

    trace = bool(os.environ.get("GNN_TRACE"))
    tmpdir = os.environ.get("GNN_TRACE_DIR") or None
    res = run_bass_kernel_spmd(nc, in_maps, core_ids=list(range(C)),
                               trace=trace, tmpdir=tmpdir)
    _LAST_RESULTS = res
    zg = res.results[0]["zg"]                    # [512, 128]
    return zg.reshape(GOUT, 8, 16).astype(np.float32)



# revision 11
# speedup vs baseline: 4.6512x; 1.0177x over previous
"""Trainium2 Bass kernel for the DGCL GNN (3 GIN conv layers + 8-factor
disentangled head + global add pool).

Self-contained: host-side numpy preprocessing (graph partitioning /
weight packing), an SPMD Bass/Tile device program for 8 NeuronCores, and
the gather/unshard glue.

Structure of the computation (mathematically identical to the reference):
  - The K=8 disentangled head factors share the same edge aggregation, and
    their per-factor MLPs concatenate into [128,128] dense / block-diagonal
    matmuls.  So the network is 5 uniform layers:
        z = h + scatter_add(gather(h, src), dst)
        v = relu(z @ W1 + b1) @ W2 + b2
        h' = BN(v) (+ relu for layers 0,1,3)
    followed by a per-graph add-pool.
  - Nodes (and their incoming edges) are sharded contiguously across the 8
    cores.  Edge gathers read a bf16 replica of h from local DRAM via
    dma_gather (int16 indices -> lo/hi half split); aggregation happens as
    bf16 one-hot matmuls accumulating in fp32 PSUM.  The self term is added
    in fp32 from an SBUF-resident transposed copy of the core's own shard.
    The h replica is refreshed each layer with an AllGather; BN statistics
    and the pooled output use AllReduce.
"""

import math
from contextlib import ExitStack

import numpy as np

import concourse.bacc as bacc
import concourse.bass as bass
import concourse.mybir as mybir
import concourse.tile as tile
from concourse.bass_utils import run_bass_kernel_spmd
from concourse.masks import make_identity

P = 128
F = 128
GOUT = 512          # output graph rows (harness G = 512)
BN_EPS = 1e-5
RELU_AFTER = [True, True, False, True, False]
f32 = mybir.dt.float32
bf16 = mybir.dt.bfloat16
i16 = mybir.dt.int16


class Cfg:
    def __init__(self, N, C, Th, group_blocks=5):
        self.N = N                      # real node count
        self.C = C                      # cores
        self.Nshard = -(-N // (C * P)) * P
        self.Np = self.Nshard * C
        self.B = self.Nshard // P       # dst blocks per core
        self.Th = Th                    # 128-edge tiles per block-half
        self.Gb = group_blocks          # blocks per gather-call group
        self.n_groups = -(-self.B // group_blocks)
        self.H = self.Np // 2
        self.total_tiles = self.B * 2 * Th
        self.tile_base = None           # filled by prep_host / fill_groups
        self.nb_of_group = None
        self.no_cc = False
        self.layers = (0, 1, 2, 3, 4)   # which layer bodies to emit
        self.skip_agg = 0               # 1: no onehot/agg-mm; 2: no gathers too

    def fill_groups(self):
        tb, bases, nbs = 0, [], []
        for g in range(self.n_groups):
            nb = min((g + 1) * self.Gb, self.B) - g * self.Gb
            bases.append(tb)
            nbs.append(nb)
            tb += 2 * nb * self.Th
        self.tile_base, self.nb_of_group = bases, nbs
        return self


# ----------------------------------------------------------------------------
# Host-side preprocessing
# ----------------------------------------------------------------------------

def _blockdiag(W):
    K_, d_, _ = W.shape
    out = np.zeros((K_ * d_, K_ * d_), np.float32)
    for k in range(K_):
        out[k * d_:(k + 1) * d_, k * d_:(k + 1) * d_] = W[k]
    return out


def prep_host(x, edge_index, batch, params, C=8, group_blocks=5):
    """Returns (cfg, in_maps) ready for the device program."""
    x = np.asarray(x, np.float32)
    N = x.shape[0]
    src = np.asarray(edge_index[0], np.int64)
    dst = np.asarray(edge_index[1], np.int64)
    batch = np.asarray(batch, np.int64)

    order = np.argsort(dst, kind="stable")
    s_sorted = src[order]
    d_sorted = dst[order]

    gblock = d_sorted // P                      # global dst block id
    slot = (d_sorted % P).astype(np.float32)

    NshardP = -(-N // (C * P)) * P
    B = NshardP // P
    Np = NshardP * C
    H = Np // 2
    assert H <= 32767 and Np - H <= 32767, "half-table exceeds int16 range"

    # split each block's edge list by source half (int16 index limit)
    is_hi = (s_sorted >= H).astype(np.int64)
    halfkey = gblock * 2 + is_hi
    counts2 = np.bincount(halfkey, minlength=C * B * 2)
    Th = max(1, int(math.ceil(counts2.max() / P)))
    cfg = Cfg(N, C, Th, group_blocks).fill_groups()
    Gb = group_blocks

    total_tiles = cfg.total_tiles
    idx16 = np.zeros((C, 16, total_tiles * 8), np.int16)
    slot_arr = np.full((C, 128, total_tiles), 999.0, np.float32)

    order2 = np.argsort(halfkey, kind="stable")
    s2 = s_sorted[order2]
    slot2 = slot[order2]
    startpos = np.zeros(C * B * 2 + 1, np.int64)
    startpos[1:] = np.cumsum(counts2)

    for c in range(C):
        for b in range(B):
            g = b // Gb
            bi = b - g * Gb
            nb = cfg.nb_of_group[g]
            for hf in range(2):
                k = (c * B + b) * 2 + hf
                cnt = counts2[k]
                if cnt == 0:
                    continue
                sl = slice(startpos[k], startpos[k + 1])
                j = np.arange(cnt)
                call_tile0 = cfg.tile_base[g] + (nb * Th if hf else 0)
                gt = call_tile0 + bi * Th + j // P
                slot_arr[c, j % P, gt] = slot2[sl]
                vals = s2[sl] - (H if hf else 0)
                gcol = (call_tile0 + bi * Th) * 8 + j // 16
                idx16[c, j % 16, gcol] = vals.astype(np.int16)
    idx16 = np.tile(idx16, (1, 8, 1))           # replicate 16-row wrap x8

    # per-node batch selector (999 sentinel for padding nodes)
    ids = np.arange(Np)
    bsel = np.where(ids < N, batch[np.minimum(ids, N - 1)], 999).astype(np.float32)
    batchsel = bsel.reshape(C, B, P).transpose(0, 2, 1).copy()   # [C, P, B]

    npad = np.array(
        [max(0, (c + 1) * NshardP - max(N, c * NshardP)) for c in range(C)],
        np.float32,
    )

    # padded node features (zeros for pad rows) + bf16 gather replica
    x_pad = np.zeros((Np, F), np.float32)
    x_pad[:N] = x
    xb = x_pad.astype(mybir.dt.np(bf16))

    # --- weights ----------------------------------------------------------
    gc_W1 = np.asarray(params["gc_W1"], np.float32)
    gc_W2 = np.asarray(params["gc_W2"], np.float32)
    gc_b1 = np.asarray(params["gc_b1"], np.float32)
    gc_b2 = np.asarray(params["gc_b2"], np.float32)
    gc_g = np.asarray(params["gc_g"], np.float32)
    gc_be = np.asarray(params["gc_be"], np.float32)
    h0_W1 = np.asarray(params["h0_W1"], np.float32)
    h0_W2 = np.asarray(params["h0_W2"], np.float32)
    h1_W1 = np.asarray(params["h1_W1"], np.float32)
    h1_W2 = np.asarray(params["h1_W2"], np.float32)

    W1s = [gc_W1[0], gc_W1[1], gc_W1[2],
           h0_W1.transpose(1, 0, 2).reshape(F, F), _blockdiag(h1_W1)]
    W2s = [gc_W2[0], gc_W2[1], gc_W2[2], _blockdiag(h0_W2), _blockdiag(h1_W2)]
    b1s = [gc_b1[0], gc_b1[1], gc_b1[2],
           np.asarray(params["h0_b1"], np.float32).reshape(-1),
           np.asarray(params["h1_b1"], np.float32).reshape(-1)]
    b2s = [gc_b2[0], gc_b2[1], gc_b2[2],
           np.asarray(params["h0_b2"], np.float32).reshape(-1),
           np.asarray(params["h1_b2"], np.float32).reshape(-1)]
    gs = [gc_g[0], gc_g[1], gc_g[2],
          np.asarray(params["h0_g"], np.float32).reshape(-1),
          np.asarray(params["h1_g"], np.float32).reshape(-1)]
    bes = [gc_be[0], gc_be[1], gc_be[2],
           np.asarray(params["h0_be"], np.float32).reshape(-1),
           np.asarray(params["h1_be"], np.float32).reshape(-1)]

    wpack = np.concatenate(
        sum(([W1s[l], W2s[l]] for l in range(5)), []), axis=1
    ).astype(np.float32)                                        # [F, 1280]
    bpack = np.stack(
        sum(([b1s[l], b2s[l], gs[l], bes[l]] for l in range(5)), []), axis=1
    ).astype(np.float32)                                        # [F, 20]

    iota128 = np.tile(np.arange(P, dtype=np.float32), (P, 1)).astype(
        mybir.dt.np(bf16))
    iota512 = np.tile(np.arange(GOUT, dtype=np.float32), (P, 1))

    # host-precomputed one-hot tiles (graph-static, shared by all layers):
    # oh[p, t*P + s] = 1 iff slot_arr[p, t] == s  (999 pad rows stay zero)
    ohs = []
    for c in range(C):
        sl = slot_arr[c].astype(np.int32)               # [P, total_tiles]
        oh = np.zeros((P, total_tiles, P), np.float32)
        pp, tt = np.nonzero(sl < P)
        oh[pp, tt, sl[pp, tt]] = 1.0
        ohs.append(np.ascontiguousarray(
            oh.reshape(P, total_tiles * P)).astype(mybir.dt.np(bf16)))

    # host-precomputed pooling one-hots (batch-static, layer 4 only):
    # poh[p, b*GOUT + g] = 1 iff batchsel[p, b] == g  (999 pad rows stay 0)
    pohs = []
    for c in range(C):
        bs = batchsel[c].astype(np.int32)               # [P, B]
        ph = np.zeros((P, B, GOUT), np.float32)
        pp, bb2 = np.nonzero(bs < GOUT)
        ph[pp, bb2, bs[pp, bb2]] = 1.0
        pohs.append(np.ascontiguousarray(ph.reshape(P, B * GOUT)))

    in_maps = []
    for c in range(C):
        in_maps.append({
            "xb": xb,
            "xown": np.ascontiguousarray(
                x_pad[c * NshardP:(c + 1) * NshardP]),
            "idx": idx16[c],
            "slot": slot_arr[c],
            "batchsel": batchsel[c],
            "npad": np.full((P, 1), npad[c], np.float32),
            "wpack": wpack,
            "bpack": bpack,
            "iota128": iota128,
            "iota512": iota512,
            "oh": ohs[c],
            "poh": pohs[c],
        })
    return cfg, in_maps


# ----------------------------------------------------------------------------
# Device program
# ----------------------------------------------------------------------------

def build_device(tc, io, cfg):
    nc = tc.nc
    C, B, Th, Gb = cfg.C, cfg.B, cfg.Th, cfg.Gb
    rg = [list(range(C))]
    AF = mybir.ActivationFunctionType
    OP = mybir.AluOpType
    no_cc = cfg.no_cc

    # internal DRAM ("Shared" outputs only supported for >4-core groups)
    sh = "Shared" if C > 4 else "Local"
    h_dram = nc.dram_tensor("h_rep", [cfg.Np, F], bf16, kind="Internal",
                            addr_space=sh)
    vsh_dram = nc.dram_tensor("v_shard", [cfg.Nshard, F], bf16, kind="Internal")
    st_in = nc.dram_tensor("st_in", [P, 2], f32, kind="Internal")
    st_out = nc.dram_tensor("st_out", [P, 2], f32, kind="Internal",
                            addr_space=sh)
    pl_in = nc.dram_tensor("pl_in", [P, GOUT], f32, kind="Internal")
    pl_out = nc.dram_tensor("pl_out", [P, GOUT], f32, kind="Internal",
                            addr_space=sh)

    ctx = ExitStack()
    cpool = ctx.enter_context(tc.tile_pool(name="consts", bufs=1))
    gpool = ctx.enter_context(tc.tile_pool(name="gather", bufs=2))
    opool = ctx.enter_context(tc.tile_pool(name="onehot", bufs=4))
    zpool = ctx.enter_context(tc.tile_pool(name="work", bufs=3))
    spool = ctx.enter_context(tc.tile_pool(name="small", bufs=3))
    ps_agg_pool = ctx.enter_context(tc.tile_pool(name="ps_agg", bufs=2, space="PSUM"))
    ps_mlp_pool = ctx.enter_context(tc.tile_pool(name="ps_mlp", bufs=2, space="PSUM"))
    ps_tr_pool = ctx.enter_context(tc.tile_pool(name="ps_tr", bufs=2, space="PSUM"))
    ps_pool512 = ctx.enter_context(tc.tile_pool(name="ps_p512", bufs=1, space="PSUM"))

    # resident constants / state
    idx_sb = cpool.tile([P, cfg.total_tiles * 8], i16, tag="idx")
    bsel_sb = cpool.tile([P, B], f32, tag="bsel")
    npad_sb = cpool.tile([P, 1], f32, tag="npad")
    w_sb = cpool.tile([P, 10 * F], f32, tag="w")
    bb_sb = cpool.tile([P, 20], f32, tag="bb")
    io128_sb = cpool.tile([P, P], bf16, tag="io128")
    io512_sb = cpool.tile([P, GOUT], f32, tag="io512")
    vT_sb = cpool.tile([P, B * F], f32, tag="vT")
    hTown_sb = cpool.tile([P, B * F], f32, tag="hTown")
    ssum_sb = cpool.tile([P, B], f32, tag="ssum")
    ssq_sb = cpool.tile([P, B], f32, tag="ssq")
    ident_sb = cpool.tile([P, P], f32, tag="ident")
    hpad_sb = cpool.tile([P, 1], f32, tag="hpad")

    nc.sync.dma_start(out=idx_sb[:], in_=io["idx"][:])
    nc.sync.dma_start(out=bsel_sb[:], in_=io["batchsel"][:])
    nc.sync.dma_start(out=npad_sb[:], in_=io["npad"][:])
    nc.sync.dma_start(out=w_sb[:], in_=io["wpack"][:])
    nc.sync.dma_start(out=bb_sb[:], in_=io["bpack"][:])
    nc.sync.dma_start(out=io128_sb[:], in_=io["iota128"][:])
    nc.sync.dma_start(out=io512_sb[:], in_=io["iota512"][:])
    make_identity(nc, ident_sb[:])
    nc.vector.memset(hpad_sb[:], 0.0)
    from concourse import library_config
    nc.gpsimd.load_library(library_config.mlp)

    # hTown <- x_own^T (fp32 self-term, transposed layout)
    for b in range(B):
        xo = zpool.tile([P, F], f32, tag="xo")
        nc.sync.dma_start(out=xo[:], in_=io["xown"][b * P:(b + 1) * P, :])
        ps_x = ps_tr_pool.tile([P, F], f32, tag="tr")
        nc.tensor.transpose(ps_x[:], xo[:], ident_sb[:])
        nc.any.tensor_copy(out=hTown_sb[:, b * F:(b + 1) * F], in_=ps_x[:])

    for l in cfg.layers:
        w1 = w_sb[:, l * 2 * F:(l * 2 + 1) * F]
        w2 = w_sb[:, (l * 2 + 1) * F:(l * 2 + 2) * F]
        b1 = bb_sb[:, 4 * l + 0:4 * l + 1]
        b2 = bb_sb[:, 4 * l + 1:4 * l + 2]
        ga = bb_sb[:, 4 * l + 2:4 * l + 3]
        be = bb_sb[:, 4 * l + 3:4 * l + 4]
        H = cfg.H
        if l == 0:
            src_lo, src_hi = io["xb"][0:H, :], io["xb"][H:cfg.Np, :]
        else:
            src_lo, src_hi = h_dram.ap()[0:H, :], h_dram.ap()[H:cfg.Np, :]

        # ---- gather + aggregate + MLP, blockwise --------------------------
        for g in range(cfg.n_groups):
            b_lo = g * Gb
            nb = cfg.nb_of_group[g]
            ntc = nb * Th                       # tiles per half-call
            tb0 = cfg.tile_base[g]
            gbuf = gpool.tile([P, 2 * ntc * F], bf16, tag="gbuf")
            ohb = gpool.tile([P, 2 * ntc * P], bf16, tag="ohbuf")
            nc.sync.dma_start(
                out=ohb[:], in_=io["oh"][:, tb0 * P:(tb0 + 2 * ntc) * P])
            ni = ntc * P
            if cfg.skip_agg < 2:
                for hf, src_h in ((0, src_lo), (1, src_hi)):
                    c0 = (tb0 + hf * ntc) * 8
                    nc.gpsimd.dma_gather(
                        gbuf[:, hf * ntc * F:(hf + 1) * ntc * F].rearrange(
                            "p (k d) -> p k d", d=F),
                        src_h,
                        idx_sb[:, c0:c0 + ntc * 8],
                        ni, ni, F, single_packet=False,
                        queue_num=(2 * g + hf) % 4)
            for bi in range(nb):
                b = b_lo + bi
                if cfg.skip_agg == 0:
                    ps_agg = ps_agg_pool.tile([P, F], f32, tag="agg")
                    for t in range(2 * Th):
                        hf, th = (0, t) if t < Th else (1, t - Th)
                        ltile = hf * ntc + bi * Th + th
                        nc.tensor.matmul(
                            ps_agg[:],
                            lhsT=gbuf[:, ltile * F:(ltile + 1) * F],
                            rhs=ohb[:, ltile * P:(ltile + 1) * P],
                            start=(t == 0), stop=(t == 2 * Th - 1))
                zT = zpool.tile([P, F], f32, tag="zT")
                if cfg.skip_agg == 0:
                    nc.any.tensor_tensor(
                        out=zT[:], in0=ps_agg[:],
                        in1=hTown_sb[:, b * F:(b + 1) * F], op=OP.add)
                else:
                    nc.any.tensor_tensor(
                        out=zT[:], in0=hTown_sb[:, b * F:(b + 1) * F],
                        in1=hTown_sb[:, b * F:(b + 1) * F], op=OP.add)
                ps_u = ps_mlp_pool.tile([P, F], f32, tag="mlp")
                nc.tensor.matmul(ps_u[:], lhsT=w1, rhs=zT[:], start=True, stop=True)
                uT = zpool.tile([P, F], f32, tag="uT")
                nc.scalar.activation(uT[:], ps_u[:], AF.Relu, bias=b1, scale=1.0)
                ps_v = ps_mlp_pool.tile([P, F], f32, tag="mlp")
                nc.tensor.matmul(ps_v[:], lhsT=w2, rhs=uT[:], start=True, stop=True)
                vT = vT_sb[:, b * F:(b + 1) * F]
                nc.any.tensor_scalar(
                    out=vT, in0=ps_v[:], scalar1=b2, scalar2=None, op0=OP.add,
                    op1=OP.add, accum_out=ssum_sb[:, b:b + 1])
                sq = zpool.tile([P, F], f32, tag="sq")
                nc.scalar.activation(sq[:], vT, AF.Square,
                                     accum_out=ssq_sb[:, b:b + 1])

        # ---- BN statistics (+ padding-node correction) --------------------
        s1 = spool.tile([P, 1], f32, tag="s1")
        s2 = spool.tile([P, 1], f32, tag="s2")
        nc.vector.reduce_sum(s1[:], ssum_sb[:], axis=mybir.AxisListType.X)
        nc.vector.reduce_sum(s2[:], ssq_sb[:], axis=mybir.AxisListType.X)
        # v_pad = W2.T@relu(W1.T@hpad + b1) + b2  (value of every pad node)
        ps_zp = ps_tr_pool.tile([P, 1], f32, tag="tr")
        nc.tensor.matmul(ps_zp[:], lhsT=w1, rhs=hpad_sb[:], start=True, stop=True)
        upad = spool.tile([P, 1], f32, tag="upad")
        nc.scalar.activation(upad[:], ps_zp[:], AF.Relu, bias=b1, scale=1.0)
        ps_vp = ps_tr_pool.tile([P, 1], f32, tag="tr")
        nc.tensor.matmul(ps_vp[:], lhsT=w2, rhs=upad[:], start=True, stop=True)
        vpad = spool.tile([P, 1], f32, tag="vpad")
        nc.any.tensor_scalar(out=vpad[:], in0=ps_vp[:], scalar1=b2,
                             scalar2=None, op0=OP.add)
        t1 = spool.tile([P, 1], f32, tag="t1")
        nc.any.tensor_tensor(out=t1[:], in0=vpad[:], in1=npad_sb[:], op=OP.mult)
        nc.any.tensor_tensor(out=s1[:], in0=s1[:], in1=t1[:], op=OP.subtract)
        vpad2 = spool.tile([P, 1], f32, tag="vpad2")
        nc.scalar.activation(vpad2[:], vpad[:], AF.Square)
        t2 = spool.tile([P, 1], f32, tag="t2")
        nc.any.tensor_tensor(out=t2[:], in0=vpad2[:], in1=npad_sb[:], op=OP.mult)
        nc.any.tensor_tensor(out=s2[:], in0=s2[:], in1=t2[:], op=OP.subtract)

        stt = spool.tile([P, 2], f32, tag="stt")
        nc.any.tensor_copy(out=stt[:, 0:1], in_=s1[:])
        nc.any.tensor_copy(out=stt[:, 1:2], in_=s2[:])
        nc.sync.dma_start(out=st_in.ap(), in_=stt[:])
        if not no_cc:
            nc.gpsimd.collective_compute(
                "AllReduce", OP.add, replica_groups=rg,
                ins=[st_in.ap()], outs=[st_out.ap()])
            st_res = st_out
        else:
            st_res = st_in
        stg = spool.tile([P, 2], f32, tag="stg")
        nc.sync.dma_start(out=stg[:], in_=st_res.ap())

        inv_n = 1.0 / float(cfg.N)
        mu = spool.tile([P, 1], f32, tag="mu")
        nc.any.tensor_scalar(out=mu[:], in0=stg[:, 0:1], scalar1=inv_n,
                             scalar2=None, op0=OP.mult)
        ms = spool.tile([P, 1], f32, tag="ms")
        nc.any.tensor_scalar(out=ms[:], in0=stg[:, 1:2], scalar1=inv_n,
                             scalar2=None, op0=OP.mult)
        mu2 = spool.tile([P, 1], f32, tag="mu2")
        nc.scalar.activation(mu2[:], mu[:], AF.Square)
        var = spool.tile([P, 1], f32, tag="var")
        nc.any.tensor_tensor(out=var[:], in0=ms[:], in1=mu2[:], op=OP.subtract)
        veps = spool.tile([P, 1], f32, tag="veps")
        nc.any.tensor_scalar(out=veps[:], in0=var[:], scalar1=BN_EPS,
                             scalar2=None, op0=OP.add)
        sd = spool.tile([P, 1], f32, tag="sd")
        nc.scalar.activation(sd[:], veps[:], AF.Sqrt)
        rs = spool.tile([P, 1], f32, tag="rs")
        nc.vector.reciprocal(rs[:], sd[:])
        aa = spool.tile([P, 1], f32, tag="aa")
        nc.any.tensor_tensor(out=aa[:], in0=rs[:], in1=ga, op=OP.mult)
        mua = spool.tile([P, 1], f32, tag="mua")
        nc.any.tensor_tensor(out=mua[:], in0=mu[:], in1=aa[:], op=OP.mult)
        cc = spool.tile([P, 1], f32, tag="cc")
        nc.any.tensor_tensor(out=cc[:], in0=be, in1=mua[:], op=OP.subtract)

        # hpad' = relu?(a * v_pad + c)
        hp1 = spool.tile([P, 1], f32, tag="hp1")
        nc.any.tensor_tensor(out=hp1[:], in0=vpad[:], in1=aa[:], op=OP.mult)
        if RELU_AFTER[l]:
            hp2 = spool.tile([P, 1], f32, tag="hp2")
            nc.any.tensor_tensor(out=hp2[:], in0=hp1[:], in1=cc[:], op=OP.add)
            nc.scalar.activation(hpad_sb[:], hp2[:], AF.Relu)
        else:
            nc.any.tensor_tensor(out=hpad_sb[:], in0=hp1[:], in1=cc[:],
                                 op=OP.add)

        # ---- normalize (into hTown) + transpose + (AllGather | pool) ------
        if l == 4:
            ps_pool = ps_pool512.tile([P, GOUT], f32, tag="p512")
        for b in range(B):
            nt = hTown_sb[:, b * F:(b + 1) * F]
            if RELU_AFTER[l]:
                nc.scalar.activation(nt, vT_sb[:, b * F:(b + 1) * F],
                                     AF.Relu, bias=cc[:], scale=aa[:])
            else:
                nc.any.tensor_scalar(
                    out=nt, in0=vT_sb[:, b * F:(b + 1) * F],
                    scalar1=aa[:], scalar2=cc[:], op0=OP.mult, op1=OP.add)
            ps_t = ps_tr_pool.tile([P, F], f32, tag="tr")
            nc.tensor.transpose(ps_t[:], nt, ident_sb[:])
            if l < 4:
                ntr = zpool.tile([P, F], bf16, tag="ntr")
                nc.any.tensor_copy(out=ntr[:], in_=ps_t[:])
                nc.sync.dma_start(out=vsh_dram.ap()[b * P:(b + 1) * P, :],
                                  in_=ntr[:])
            else:
                ntr = zpool.tile([P, F], f32, tag="ntrf")
                nc.any.tensor_copy(out=ntr[:], in_=ps_t[:])
                poh = opool.tile([P, GOUT], f32, tag="poh")
                nc.sync.dma_start(
                    out=poh[:], in_=io["poh"][:, b * GOUT:(b + 1) * GOUT])
                nc.tensor.matmul(ps_pool[:], lhsT=ntr[:], rhs=poh[:],
                                 start=(b == 0), stop=(b == B - 1))
        if l < 4:
            if not no_cc:
                nc.gpsimd.collective_compute(
                    "AllGather", OP.bypass, replica_groups=rg,
                    ins=[vsh_dram.ap()], outs=[h_dram.ap()])
            else:
                nc.sync.dma_start(out=h_dram.ap()[0:cfg.Nshard, :],
                                  in_=vsh_dram.ap())

    # ---- pooled AllReduce + transpose out ---------------------------------
    pooledT = cpool.tile([P, GOUT], f32, tag="pooledT")
    nc.any.tensor_copy(out=pooledT[:], in_=ps_pool[:])
    nc.sync.dma_start(out=pl_in.ap(), in_=pooledT[:])
    if not no_cc:
        nc.gpsimd.collective_compute(
            "AllReduce", OP.add, replica_groups=rg,
            ins=[pl_in.ap()], outs=[pl_out.ap()])
        pl_res = pl_out
    else:
        pl_res = pl_in
    plr = cpool.tile([P, GOUT], f32, tag="plr")
    nc.sync.dma_start(out=plr[:], in_=pl_res.ap())
    for q in range(GOUT // P):
        ps_q = ps_tr_pool.tile([P, P], f32, tag="tr")
        nc.tensor.transpose(ps_q[:], plr[:, q * P:(q + 1) * P], ident_sb[:])
        oq = zpool.tile([P, P], f32, tag="oq")
        nc.any.tensor_copy(out=oq[:], in_=ps_q[:])
        nc.sync.dma_start(out=io["zg"][q * P:(q + 1) * P, :], in_=oq[:])
    ctx.close()


# ----------------------------------------------------------------------------
# Entry point
# ----------------------------------------------------------------------------

_CACHE = {}
_LAST_RESULTS = None


def _build_full(cfg):
    nc = bacc.Bacc("TRN2", target_bir_lowering=False, debug=False,
                   num_devices=cfg.C, num_swdge_queues=4)
    io = {}
    io["xb"] = nc.dram_tensor("xb", [cfg.Np, F], bf16,
                              kind="ExternalInput").ap()
    io["xown"] = nc.dram_tensor("xown", [cfg.Nshard, F], f32,
                                kind="ExternalInput").ap()
    io["idx"] = nc.dram_tensor("idx", [P, cfg.total_tiles * 8], i16,
                               kind="ExternalInput").ap()
    io["slot"] = nc.dram_tensor("slot", [P, cfg.total_tiles], f32,
                                kind="ExternalInput").ap()
    io["batchsel"] = nc.dram_tensor("batchsel", [P, cfg.B], f32,
                                    kind="ExternalInput").ap()
    io["npad"] = nc.dram_tensor("npad", [P, 1], f32, kind="ExternalInput").ap()
    io["wpack"] = nc.dram_tensor("wpack", [P, 10 * F], f32,
                                 kind="ExternalInput").ap()
    io["bpack"] = nc.dram_tensor("bpack", [P, 20], f32,
                                 kind="ExternalInput").ap()
    io["iota128"] = nc.dram_tensor("iota128", [P, P], bf16,
                                   kind="ExternalInput").ap()
    io["iota512"] = nc.dram_tensor("iota512", [P, GOUT], f32,
                                   kind="ExternalInput").ap()
    io["oh"] = nc.dram_tensor("oh", [P, cfg.total_tiles * P], bf16,
                              kind="ExternalInput").ap()
    io["poh"] = nc.dram_tensor("poh", [P, cfg.B * GOUT], f32,
                               kind="ExternalInput").ap()
    io["zg"] = nc.dram_tensor("zg", [GOUT, F], f32, kind="ExternalOutput").ap()
    with tile.TileContext(nc) as tc:
        build_device(tc, io, cfg)
    nc.compile()
    return nc


def kernel(**inputs):
    import os
    global _LAST_RESULTS
    x = np.asarray(inputs["x"], np.float32)
    edge_index = np.asarray(inputs["edge_index"])
    batch = np.asarray(inputs["batch"])
    C = 8
    cfg, in_maps = prep_host(x, edge_index, batch, inputs, C=C)

    key = (x.shape, edge_index.shape, cfg.Th)
    if key not in _CACHE:
        _CACHE[key] = _build_full(cfg)
    nc = _CACHE[key]

    trace = bool(os.environ.get("GNN_TRACE"))
    tmpdir = os.environ.get("GNN_TRACE_DIR") or None
    res = run_bass_kernel_spmd(nc, in_maps, core_ids=list(range(C)),
                               trace=trace, tmpdir=tmpdir)
    _LAST_RESULTS = res
    zg = res.results[0]["zg"]                    # [512, 128]
    return zg.reshape(GOUT, 8, 16).astype(np.float32)

